# revision 36
# baseline (speedup 1.0000x reference)
"""Trainium2 Bass kernel for nn_AltAttention (dense transformer attention block).

Reference computation (B=4, S=2048, D=512, H=8, Dh=64):
    qkv  = hidden @ W_qkv + b_qkv                      -> q, k, v per head
    attn = softmax(q k^T * D**-0.5 + alibi, masked)
    out  = (attn @ v) @ W_proj + b_proj

Sharding: one head per NeuronCore (8 heads = 8 cores). Each core computes
q/k/v for its head from the full hidden states, runs attention with
transposed score tiles [ks, qs], applies the proj matmul on-chip, and
writes a partial projection output (fp16). The host sums the 8 partials
(the tensor-parallel all-reduce) to form the output.

Performance structure (HW-calibrated on trn2 via microbenchmarks):
  - exp(s + alibi) = exp(s) * exp(alibi): exp(alibi) is precomputed on the
    host (transposed + tiled, bf16) so the on-chip alibi add becomes a
    2x-mode bf16 DVE multiply instead of a 1x fp32 add.
  - q^T and k^T are stored twice along the partition dim; score matmuls run
    as ROW-TILED K=64 pairs (tile_position (0,0)/(64,0)) that execute
    concurrently on disjoint halves of the PE array (192 ns/pair measured
    vs 345 serial).
  - v projection matmuls run as COL-TILED M=64 pairs.
  - attn@V uses a FULL 128-column stationary operand (zero-padded vaug:
    cols 0:32+33:64 zero, col 32 ones for the softmax sums, cols 64:128 =
    v^T) -- a 128-col weight load enables FWL and measures 267 ns/matmul
    vs ~500+ for a 65-col load. Softmax sums land on x_ps row 32 and
    attn@V output on rows 64:128 (32-aligned PSUM partition bases).
  - 1/sum is DEFERRED past the proj: the full [128,512] x_ps is evicted
    unnormalized in ONE bf16 copy (rows outside sums/x are exact zeros and
    wproj_aug zero-pads the matching K=128 rows), the sums row is
    transposed to per-partition layout with 4 tiny PE transposes into the
    dead x_ps bank, one [128,4] DVE reciprocal replaces the 3.1us [1,512]
    one (reciprocal runs at 8 cycles/element), and the scale rides the
    proj-output eviction as a per-partition tensor_scalar multiply. The
    proj bias rides row 32 of W_proj_aug (core 0 carries b_proj; every
    core also carries bv @ W_h, exact because sum(attn)=1). Phase-1 v
    col-pairs evict as ONE [128,512] copy each.
  - ISSUE-ORDER SOFTWARE PIPELINING: engines execute their queues in
    program order, so attn@V lags its score-group by AV_LAG=3 and each
    block's normalize+proj tail is issued one full block later -- by the
    time an engine reaches a dependent instruction its inputs are long
    since ready. This alone took phase 2 from ~349us to ~221us. The
    exp(alibi) multiplies run pairwise (one FD=2048 DVE op per two score
    groups) to halve DVE op count; the hidden-state/alibi DMA pools live
    at the outer scope so consecutive invocations pipeline (steady-state
    marginal cost ~205us/rep).
  - qkv biases fold into the PSUM->SBUF eviction as per-partition
    tensor_scalar adds on DVE (measured faster than ACT Identity+bias);
    the v bias folds into the proj bias row.
  - partial outputs are fp16 (half the output DMA traffic); host
    accumulates in fp32.

Measured: ~212 us/rep steady-state (repeat-slope, R=9..33, median of
per-trial slopes) vs 469 us baseline; max rel err 0.0047 (tol 2e-2).
"""

import sys

sys.path.insert(0, "/opt/trn_rl_repo")

import numpy as np
import ml_dtypes

import concourse.bass as bass
import concourse.tile as tile
from concourse import bacc, mybir
from concourse.bass_utils import run_bass_kernel_spmd

BF16 = mybir.dt.bfloat16
F16 = mybir.dt.float16
F32 = mybir.dt.float32
NP_BF16 = ml_dtypes.bfloat16

B, S, D, H = 4, 2048, 512, 8
Dh = D // H  # 64
BS = B * S  # 8192
P = 128
NKT = S // P  # 16 ks tiles per batch
NQB = S // 512  # 4 query blocks of 512 per batch
SCALE = D ** (-0.5)
ACT = mybir.ActivationFunctionType
I16 = mybir.dt.int16
# Schraudolph bf16 fast-exp constants: bits = round(C1*(s+a) + C2);
# C2/C1 is pre-added to the alibi on the host (fp16), the +0.5 makes the
# int16 convert round-to-nearest if the hardware truncates
SCH_C1 = np.float32(128.0 / np.log(2.0))
SCH_C2_OVER_C1 = np.float32((127.0 - 0.0434) * np.log(2.0))
# number of leading score-groups (of 8 per query-block) whose exp(alibi)
# multiply runs on GPSIMD instead of DVE
GP_EA = 0


def build_program(eb: int, repeat: int = 1, phases=(1, 2), skel=False,
                  gp_ea=None, bcast="pe", rowtile=True, elem="both",
                  p1act=False, p1tpos="end", hbufs=2, av_lag=3, tail_g=3,
                  sfd=1024, pbufs=2, sch=0):
    """Build the per-core Bass program. eb = number of exp-alibi slices
    (1 when the attention mask is all ones, B otherwise)."""
    if gp_ea is None:
        gp_ea = GP_EA
    if eb != 1:
        sch = 0  # additive-alibi fast-exp has no mask clamp; exp path only
    nc = bacc.Bacc("TRN2", target_bir_lowering=False, debug=False, num_devices=H)

    hiddenT = nc.dram_tensor("hiddenT", [D, BS], BF16, kind="ExternalInput")
    # ea layout: [eb, NQB, 128, NKT, 512] so each (e, qb) slice is one
    # contiguous 2 MB DMA
    ea = nc.dram_tensor("ea", [eb, NQB, P, NKT, 512], BF16,
                        kind="ExternalInput")
    wqk = nc.dram_tensor("wqk", [4, P, P], BF16, kind="ExternalInput")
    bqk = nc.dram_tensor("bqk", [P, 1], F32, kind="ExternalInput")
    wv = nc.dram_tensor("wv", [4, P, Dh], BF16, kind="ExternalInput")
    wproj = nc.dram_tensor("wproj", [P, D], BF16, kind="ExternalInput")
    part = nc.dram_tensor("part", [BS, D], F16, kind="ExternalOutput")

    hT_re = hiddenT[:].rearrange("(c p) s -> p c s", p=P)  # [128, 4, 8192]

    with tile.TileContext(nc) as tc:
        with tc.tile_pool(name="consts", bufs=1) as consts, \
             tc.tile_pool(name="persist", bufs=1) as persist, \
             tc.tile_pool(name="hpool", bufs=hbufs) as hpool, \
             tc.tile_pool(name="eapool", bufs=2) as eapool:
            wqk_sb = consts.tile([P, 4, P], BF16)
            nc.sync.dma_start(wqk_sb[:], wqk[:].rearrange("c p m -> p c m"))
            wv_sb = consts.tile([P, 4, Dh], BF16)
            nc.sync.dma_start(wv_sb[:], wv[:].rearrange("c p m -> p c m"))
            bqk_sb = consts.tile([P, 1], F32)
            nc.sync.dma_start(bqk_sb[:], bqk[:])
            wproj_sb = consts.tile([P, D], BF16)
            nc.sync.dma_start(wproj_sb[:], wproj[:])
            ident_bf = consts.tile([P, 1], BF16)
            nc.vector.memset(ident_bf[:], 1.0)

            # qk2 rows: 0:64 = qT, 64:128 = kT (matmul-native -> single
            # eviction); kq2 is the partition-swapped copy so row-tiled
            # score pairs find k/q on both PE halves
            qk2 = persist.tile([P, BS], BF16)
            kq2 = persist.tile([P, BS], BF16)
            # padded layout: tile t = [:, t, 63:128]; col 63 = ones (sums row),
            # cols 64:128 = v^T (DMA-transpose needs 128B-aligned dest offsets)
            vaug = persist.tile([P, B * NKT, P], BF16)
            nc.vector.memset(vaug[:], 0.0)
            # ones column at 32 -> softmax sums land on x_ps row 32 (the
            # 32-aligned base PSUM reads need); v^T occupies cols 64:128
            nc.vector.memset(vaug[:, :, 32:33], 1.0)
            # [128, 8 x 512]: col-tiled pairs stacked on 128 partitions
            vt_all = persist.tile([P, B * 2, 512], BF16)

            if skel:
                p_fix = persist.tile([P, NKT, 512], BF16)
                nc.vector.memset(p_fix[:], 0.01)
                xs_fix = persist.tile([P, 512], BF16)
                nc.vector.memset(xs_fix[:], 0.02)
            if 1 not in phases:
                nc.vector.memset(qk2[:], 0.01)
                nc.vector.memset(kq2[:], 0.01)
                nc.vector.memset(vaug[:], 0.01)
                nc.vector.memset(vaug[:, :, 32:33], 1.0)

            for rep in range(repeat):
                # ---------------- phase 1: qkv projections ----------------
                if 1 in phases:
                 with tc.tile_pool(name="qkps", bufs=4, space="PSUM") as qkps, \
                      tc.tile_pool(name="vtps", bufs=3, space="PSUM") as vtps:
                    for b in range(B):
                        ht = hpool.tile([P, 4, S], BF16)
                        nc.sync.dma_start(ht[:],
                                          hT_re[:, :, b * S : (b + 1) * S])
                        for sci in range(4):
                            csl = slice(sci * 512, (sci + 1) * 512)
                            col0 = b * S + sci * 512

                            qk_ps = qkps.tile([P, 512], F32)
                            for c in range(4):
                                nc.tensor.matmul(qk_ps[:], wqk_sb[:, c, :],
                                                 ht[:, c, csl],
                                                 start=(c == 0), stop=(c == 3))
                            sl = slice(col0, col0 + 512)
                            # ONE [128,512] eviction: bias vector already has
                            # q-bias rows 0:64 and k-bias rows 64:128
                            nc.vector.tensor_scalar_add(
                                qk2[:, sl], qk_ps[:], bqk_sb[:])

                        # v: col-tiled M=64 pairs -- chunk 2p -> partitions
                        # 0:64, chunk 2p+1 -> partitions 64:128, concurrent
                        for pair in range(2):
                            sla = slice(pair * 1024, pair * 1024 + 512)
                            slb = slice(pair * 1024 + 512, pair * 1024 + 1024)
                            vt_ps = vtps.tile([P, 512], F32)
                            for c in range(4):
                                nc.tensor.matmul(vt_ps[0:Dh, :], wv_sb[:, c, :],
                                                 ht[:, c, sla],
                                                 start=(c == 0), stop=(c == 3),
                                                 skip_group_check=True)
                                nc.tensor.matmul(vt_ps[Dh:P, :], wv_sb[:, c, :],
                                                 ht[:, c, slb],
                                                 start=(c == 0), stop=(c == 3),
                                                 skip_group_check=True)
                            # one [128,512] eviction covers both col-tiled
                            # chunks (rows 0:64 = chunk a, 64:128 = chunk b)
                            nc.vector.tensor_copy(
                                vt_all[:, b * 2 + pair, :], vt_ps[:])

                        bsl = slice(b * S, (b + 1) * S)
                        nc.sync.dma_start(kq2[0:Dh, bsl], qk2[Dh:P, bsl])
                        nc.sync.dma_start(kq2[Dh:P, bsl], qk2[0:Dh, bsl])
                    if p1tpos == "end":
                        # all transposes back-to-back: a single XBAR-mode
                        # transition on the DMA path instead of one per chunk.
                        # pair j covers chunks 2j (rows 0:64) / 2j+1 (64:128)
                        for j in range(B * 2):
                            nc.sync.dma_start(
                                vaug[:, j * 8 : j * 8 + 4, Dh:P],
                                vt_all[0:Dh, j, :], transpose=True)
                            nc.sync.dma_start(
                                vaug[:, j * 8 + 4 : j * 8 + 8, Dh:P],
                                vt_all[Dh:P, j, :], transpose=True)

                # ---------------- phase 2: attention + proj ----------------
                if 2 in phases:
                 with tc.tile_pool(name="ppool", bufs=pbufs) as ppool, \
                      tc.tile_pool(name="upool", bufs=2) as upool, \
                      tc.tile_pool(name="xspool", bufs=2) as xspool, \
                      tc.tile_pool(name="rsbpool", bufs=2) as rsbpool, \
                      tc.tile_pool(name="recpool", bufs=2) as recpool, \
                      tc.tile_pool(name="outpool", bufs=2) as outpool, \
                      tc.tile_pool(name="spool", bufs=(1 if sfd == 2048 else 2), space="PSUM") as spool, \
                      tc.tile_pool(name="xpool", bufs=2, space="PSUM") as xpool, \
                      tc.tile_pool(name="ops", bufs=2, space="PSUM") as ops:
                    # software-pipelined issue order: attn@V lags its
                    # score-group by AV_LAG so the in-order PE queue never
                    # waits on ACT/DVE; each block's normalize+proj tail is
                    # issued one full block later for the same reason.
                    AV_LAG = av_lag

                    def emit_tail(x_ps, qb, b):
                        # x_ps rows: 0:63 zero, 63 sums, 64:128 attn@V out.
                        # Evict UNNORMALIZED xs (bf16); 1/sum is applied
                        # per-partition at the proj output instead.
                        # ONE full-height eviction: xs row 32 = sums, rows
                        # 64:128 = attn@V out, everything else exact zeros
                        # (vaug cols 0:32+33:64 are zero). wproj_aug rows
                        # [0s;brow;0s;W_h] make the K=128 proj equivalent.
                        xs_t = xspool.tile([P, 512], BF16)
                        nc.vector.tensor_copy(xs_t[:], x_ps[:])
                        # transpose the sums row [1,512] -> [128,4] via 4 PE
                        # transposes into the now-dead x_ps bank (bf16 view,
                        # even columns for 4B-aligned PSUM writes)
                        tp = x_ps[:, 0:4].bitcast(BF16)  # [128, 8] bf16 view
                        for m in range(4):
                            nc.tensor.matmul(
                                tp[:, 2 * m : 2 * m + 1],
                                xs_t[32:33, m * P : (m + 1) * P],
                                ident_bf[32:33, :],
                                is_transpose=True, start=True, stop=True,
                                skip_group_check=True)
                        recip_t = recpool.tile([P, 4], F32)
                        nc.vector.reciprocal(
                            recip_t[:],
                            tp.rearrange("p (a b) -> p a b", b=2)[:, :, 0])
                        out_sb = outpool.tile([P, 4, 512], F16)
                        for m in range(4):
                            out_ps = ops.tile([P, 512], F32, tag="ops")
                            nc.tensor.matmul(out_ps[:],
                                             xs_t[:, m * P : (m + 1) * P],
                                             wproj_sb[:], start=True, stop=True)
                            nc.vector.tensor_scalar_mul(
                                out_sb[:, m, :], out_ps[:],
                                recip_t[:, m : m + 1])
                        row0 = b * S + qb * 512
                        nc.sync.dma_start(
                            part[row0 : row0 + 512, :].rearrange(
                                "(m p) d -> p m d", p=P),
                            out_sb[:])

                    pending_tail = None
                    for qb in range(NQB):
                        if eb == 1:
                            ea_t = eapool.tile([P, NKT, 512], BF16)
                            nc.sync.dma_start(ea_t[:], ea[0, qb])
                        for b in range(B):
                            if eb != 1:
                                ea_t = eapool.tile([P, NKT, 512], BF16)
                                nc.sync.dma_start(ea_t[:], ea[b, qb])
                            qsl = slice(b * S + qb * 512, b * S + (qb + 1) * 512)
                            x_ps = xpool.tile([P, 512], F32)
                            p_all = ppool.tile([P, NKT, 512], BF16)

                            def attn_v(g, first, last):
                                for j in range(2):
                                    tk = g * 2 + j
                                    t = b * NKT + tk
                                    nc.tensor.matmul(
                                        x_ps[:], vaug[:, t, :],
                                        (p_fix if skel else p_all)[:, tk, :],
                                        start=(first and j == 0),
                                        stop=(last and j == 1),
                                        skip_group_check=True)

                            NG = NKT // 2
                            for g in range(NG):
                                if sfd == 1024:
                                    s_ps = spool.tile([P, 1024], F32)
                                    soff = 0
                                else:
                                    # one [128,2048] tile = two score groups
                                    # per exp call, single-buffered (4 banks)
                                    if g % 2 == 0:
                                        s_ps = spool.tile([P, 2048], F32,
                                                          name="s2k")
                                    soff = (g % 2) * 1024
                                # row-tiled K=64 pair: j=0 on PE rows 0:63,
                                # j=1 on rows 64:127 -- concurrent
                                assert rowtile, "split layout needs rowtile"
                                for j in range(2):
                                    tk = g * 2 + j
                                    ksl = slice(b * S + tk * P,
                                                b * S + (tk + 1) * P)
                                    if j == 0:
                                        kop, qop = kq2[0:Dh, ksl], \
                                                   qk2[0:Dh, qsl]
                                    else:
                                        kop, qop = qk2[Dh:P, ksl], \
                                                   kq2[Dh:P, qsl]
                                    nc.tensor.matmul(
                                        s_ps[:, soff + j * 512 :
                                             soff + (j + 1) * 512],
                                        kop, qop, start=True, stop=True)
                                if not skel and g < sch:
                                    # Schraudolph fast-exp on DVE: the bf16
                                    # bit pattern of exp(s+a) is round(C1*
                                    # (s+a)+C2) as int16. ea holds a+C2/C1
                                    # as fp16 BITS for these tks; scores
                                    # never leave [-88, 89] so no clamp or
                                    # saturation handling is needed.
                                    psl = p_all[:, 2 * g : 2 * g + 2, :]\
                                        .rearrange("p a b -> p (a b)")
                                    easl = ea_t[:, 2 * g : 2 * g + 2, :]\
                                        .rearrange("p a b -> p (a b)")
                                    u_t = upool.tile([P, 1024], F32)
                                    nc.vector.tensor_add(
                                        u_t[:], s_ps[:], easl.bitcast(F16))
                                    nc.vector.tensor_scalar(
                                        psl.bitcast(I16), u_t[:],
                                        float(SCH_C1), 0.5,
                                        mybir.AluOpType.mult,
                                        mybir.AluOpType.add)
                                elif sfd == 1024 or g % 2 == 1:
                                    eg = g if sfd == 1024 else g - 1
                                    nt = sfd // 512
                                    psl = p_all[:, eg * 2 : eg * 2 + nt, :]\
                                        .rearrange("p a b -> p (a b)")
                                    if not skel:
                                        nc.scalar.activation(psl, s_ps[:],
                                                             ACT.Exp)
                                if not skel and g % 2 == 1 and g >= sch:
                                    # one multiply per two groups: halves the
                                    # DVE op count (+drains)
                                    lo = max(2 * g - 2, 2 * sch)
                                    p2 = p_all[:, lo : 2 * g + 2, :]\
                                        .rearrange("p a b -> p (a b)")
                                    e2 = ea_t[:, lo : 2 * g + 2, :]\
                                        .rearrange("p a b -> p (a b)")
                                    eng = (nc.gpsimd if g < gp_ea
                                           else nc.vector)
                                    eng.tensor_mul(p2, p2, e2)
                                if g >= AV_LAG:
                                    attn_v(g - AV_LAG, first=(g == AV_LAG),
                                           last=False)
                                if g == tail_g and pending_tail is not None:
                                    emit_tail(*pending_tail)
                                    pending_tail = None
                            for g in range(NG - AV_LAG, NG):
                                attn_v(g, first=False, last=(g == NG - 1))
                            if skel:
                                emit_tail_skel = None
                            pending_tail = (x_ps, qb, b)
                    if pending_tail is not None:
                        emit_tail(*pending_tail)
                        pending_tail = None

    nc.compile()
    return nc


_CACHE = {}


def _get_program(eb: int):
    key = ("prog", eb)
    if key not in _CACHE:
        _CACHE[key] = build_program(eb)
    return _CACHE[key]


def prepare_inputs(hidden_states, attention_mask, alibi_bias, W_qkv, b_qkv,
                   W_proj, b_proj):
    """Host-side prep: transposes, scale folding, exp(alibi), bf16 casts.
    Returns (in_maps, eb)."""
    hidden_states = np.asarray(hidden_states, dtype=np.float32)
    attention_mask = np.asarray(attention_mask)
    alibi_bias = np.asarray(alibi_bias, dtype=np.float32)
    W_qkv = np.asarray(W_qkv, dtype=np.float32)
    b_qkv = np.asarray(b_qkv, dtype=np.float32)
    W_proj = np.asarray(W_proj, dtype=np.float32)
    b_proj = np.asarray(b_proj, dtype=np.float32)

    # per-side scale: scores contract a single K=64 copy (row-tiled pairs)
    s_side = np.float32(np.sqrt(SCALE))

    hiddenT = np.ascontiguousarray(
        hidden_states.reshape(BS, D).T).astype(NP_BF16)

    mask_trivial = bool(attention_mask.all())
    eb = 1 if mask_trivial else B

    def ea_layout(eaT):
        # eaT [S(k), S(q)] -> [NQB, 128, NKT, 512] contiguous per qb slice
        return np.ascontiguousarray(
            eaT.reshape(NKT, P, NQB, 512).transpose(2, 1, 0, 3))

    SCH = 0  # must match build_program's sch default (mask-trivial only)
    ea_all = []
    for h in range(H):
        eaT = np.exp(alibi_bias[0, h].T).astype(NP_BF16)  # [S(k), S(q)]
        if mask_trivial:
            lay = ea_layout(eaT)
            if SCH:
                # additive form for the Schraudolph tks: (a + C2/C1) as
                # fp16, bit-stored in the bf16 tensor
                add = (alibi_bias[0, h].T + SCH_C2_OVER_C1).astype(
                    np.float16).view(NP_BF16)
                lay_add = ea_layout(add)
                lay = lay.copy()
                lay[:, :, 0 : 2 * SCH, :] = lay_add[:, :, 0 : 2 * SCH, :]
            ea_all.append(lay[None])
        else:
            me = np.where(attention_mask, 1.0, 0.0).astype(NP_BF16)  # [B, S]
            ea_all.append(np.stack(
                [ea_layout(eaT * me[bi][:, None]) for bi in range(B)]))
    in_maps = []
    for h in range(H):
        # reference reshapes qkv to (B, S, H, 3*Dh) then splits: head h's
        # q/k/v live in columns [h*3*Dh, h*3*Dh + 3*Dh)
        qs = slice(h * 3 * Dh, h * 3 * Dh + Dh)
        ks = slice(h * 3 * Dh + Dh, h * 3 * Dh + 2 * Dh)
        vs = slice(h * 3 * Dh + 2 * Dh, h * 3 * Dh + 3 * Dh)
        wqk = np.concatenate([W_qkv[:, qs], W_qkv[:, ks]], axis=1) * s_side
        bqk = np.concatenate([b_qkv[qs], b_qkv[ks]]) * s_side
        wv = W_qkv[:, vs]
        Wh = W_proj[h * Dh : (h + 1) * Dh, :]
        # proj bias row: b_proj only on core 0; every core adds bv @ W_h
        # (the v-bias contribution -- sum(attn)=1 makes it a constant)
        brow = (b_proj if h == 0 else np.zeros_like(b_proj)) + b_qkv[vs] @ Wh
        wproj_aug = np.concatenate(
            [np.zeros((32, D), np.float32), brow[None, :],
             np.zeros((31, D), np.float32), Wh], axis=0)
        in_maps.append({
            "hiddenT": hiddenT,
            "ea": ea_all[h],
            "wqk": np.ascontiguousarray(
                wqk.reshape(4, P, P).astype(NP_BF16)),
            "bqk": np.ascontiguousarray(bqk[:, None]),
            "wv": np.ascontiguousarray(wv.reshape(4, P, Dh).astype(NP_BF16)),
            "wproj": wproj_aug.astype(NP_BF16),
        })
    return in_maps, eb


def kernel(**inputs):
    in_maps, eb = prepare_inputs(**inputs)
    nc = _get_program(eb)
    res = run_bass_kernel_spmd(nc, in_maps, list(range(H)))
    out = res.results[0]["part"].astype(np.float32)
    for h in range(1, H):
        out = out + res.results[h]["part"].astype(np.float32)
    return out.reshape(B, S, D)


# revision 41
# speedup vs baseline: 2.1139x; 2.1139x over previous
"""Trainium2 Bass kernel for nn_AltAttention (dense transformer attention block).

Reference computation (B=4, S=2048, D=512, H=8, Dh=64):
    qkv  = hidden @ W_qkv + b_qkv                      -> q, k, v per head
    attn = softmax(q k^T * D**-0.5 + alibi, masked)
    out  = (attn @ v) @ W_proj + b_proj

Sharding: one head per NeuronCore (8 heads = 8 cores). Each core computes
q/k/v for its head from the full hidden states, runs attention with
transposed score tiles [ks, qs], applies the proj matmul on-chip, and
writes a partial projection output (fp16). The host sums the 8 partials
(the tensor-parallel all-reduce) to form the output.

Performance structure (HW-calibrated on trn2 via microbenchmarks):
  - exp(s + alibi) = exp(s) * exp(alibi): exp(alibi) is precomputed on the
    host (transposed + tiled, bf16) so the on-chip alibi add becomes a
    2x-mode bf16 DVE multiply instead of a 1x fp32 add.
  - q^T/k^T live in a partition-split pair of tensors (qk2 = [q;k] rows,
    kq2 = the partition-swapped copy) so the qkv bias eviction is ONE
    [128,512] tensor_scalar per chunk and row-tiled K=64 score pairs
    (tile_position (0,0)/(64,0)) still find k/q on both PE halves --
    the pairs execute concurrently on disjoint array halves (192 ns/pair
    measured vs 345 serial).
  - v projection matmuls run as COL-TILED M=64 pairs.
  - attn@V uses a FULL 128-column stationary operand (zero-padded vaug:
    cols 0:32+33:64 zero, col 32 ones for the softmax sums, cols 64:128 =
    v^T) -- a 128-col weight load enables FWL and measures 267 ns/matmul
    vs ~500+ for a 65-col load. Softmax sums land on x_ps row 32 and
    attn@V output on rows 64:128 (32-aligned PSUM partition bases).
  - 1/sum is DEFERRED past the proj: the full [128,512] x_ps is evicted
    unnormalized in ONE bf16 copy (rows outside sums/x are exact zeros and
    wproj_aug zero-pads the matching K=128 rows), the sums row is
    transposed to per-partition layout with 4 tiny PE transposes into the
    dead x_ps bank, one [128,4] DVE reciprocal replaces the 3.1us [1,512]
    one (reciprocal runs at 8 cycles/element), and the scale rides the
    proj-output eviction as a per-partition tensor_scalar multiply. The
    proj bias rides row 32 of W_proj_aug (core 0 carries b_proj; every
    core also carries bv @ W_h, exact because sum(attn)=1). Phase-1 v
    col-pairs evict as ONE [128,512] copy each.
  - ISSUE-ORDER SOFTWARE PIPELINING: engines execute their queues in
    program order, so attn@V lags its score-group by AV_LAG=4 and each
    block's normalize+proj tail is issued one full block later -- by the
    time an engine reaches a dependent instruction its inputs are long
    since ready. This alone took phase 2 from ~349us to ~221us. The
    exp(alibi) multiplies run pairwise (one FD=2048 DVE op per two score
    groups) to halve DVE op count; the hidden-state/alibi DMA pools live
    at the outer scope so consecutive invocations pipeline (steady-state
    marginal cost ~205us/rep).
  - qkv biases fold into the single PSUM->SBUF eviction as a
    per-partition tensor_scalar add on DVE; the v bias folds into the
    proj bias row.
  - partial outputs are fp16 (half the output DMA traffic); host
    accumulates in fp32.

Measured: ~210 us/rep steady-state (repeat-slope, R=9..33, median of
per-trial slopes) vs 469 us baseline; max rel err 0.0047 (tol 2e-2).
"""

import sys

sys.path.insert(0, "/opt/trn_rl_repo")

import numpy as np
import ml_dtypes

import concourse.bass as bass
import concourse.tile as tile
from concourse import bacc, mybir
from concourse.bass_utils import run_bass_kernel_spmd

BF16 = mybir.dt.bfloat16
F16 = mybir.dt.float16
F32 = mybir.dt.float32
NP_BF16 = ml_dtypes.bfloat16

B, S, D, H = 4, 2048, 512, 8
Dh = D // H  # 64
BS = B * S  # 8192
P = 128
NKT = S // P  # 16 ks tiles per batch
NQB = S // 512  # 4 query blocks of 512 per batch
SCALE = D ** (-0.5)
ACT = mybir.ActivationFunctionType
I16 = mybir.dt.int16
# Schraudolph bf16 fast-exp constants: bits = round(C1*(s+a) + C2);
# C2/C1 is pre-added to the alibi on the host (fp16), the +0.5 makes the
# int16 convert round-to-nearest if the hardware truncates
SCH_C1 = np.float32(128.0 / np.log(2.0))
SCH_C2_OVER_C1 = np.float32((127.0 - 0.0434) * np.log(2.0))
# number of leading score-groups (of 8 per query-block) whose exp(alibi)
# multiply runs on GPSIMD instead of DVE
GP_EA = 0


def build_program(eb: int, repeat: int = 1, phases=(1, 2), skel=False,
                  gp_ea=None, bcast="pe", rowtile=True, elem="both",
                  p1act=False, p1tpos="end", hbufs=3, av_lag=4, tail_g=4,
                  sfd=1024, pbufs=2, sch=0):
    """Build the per-core Bass program. eb = number of exp-alibi slices
    (1 when the attention mask is all ones, B otherwise)."""
    if gp_ea is None:
        gp_ea = GP_EA
    if eb != 1:
        sch = 0  # additive-alibi fast-exp has no mask clamp; exp path only
    nc = bacc.Bacc("TRN2", target_bir_lowering=False, debug=False, num_devices=H)

    hiddenT = nc.dram_tensor("hiddenT", [D, BS], BF16, kind="ExternalInput")
    # ea layout: [eb, NQB, 128, NKT, 512] so each (e, qb) slice is one
    # contiguous 2 MB DMA
    ea = nc.dram_tensor("ea", [eb, NQB, P, NKT, 512], BF16,
                        kind="ExternalInput")
    wqk = nc.dram_tensor("wqk", [4, P, P], BF16, kind="ExternalInput")
    bqk = nc.dram_tensor("bqk", [P, 1], F32, kind="ExternalInput")
    wv = nc.dram_tensor("wv", [4, P, Dh], BF16, kind="ExternalInput")
    wproj = nc.dram_tensor("wproj", [P, D], BF16, kind="ExternalInput")
    part = nc.dram_tensor("part", [BS, D], F16, kind="ExternalOutput")

    hT_re = hiddenT[:].rearrange("(c p) s -> p c s", p=P)  # [128, 4, 8192]

    with tile.TileContext(nc) as tc:
        with tc.tile_pool(name="consts", bufs=1) as consts, \
             tc.tile_pool(name="persist", bufs=1) as persist, \
             tc.tile_pool(name="hpool", bufs=hbufs) as hpool, \
             tc.tile_pool(name="eapool", bufs=2) as eapool:
            wqk_sb = consts.tile([P, 4, P], BF16)
            nc.sync.dma_start(wqk_sb[:], wqk[:].rearrange("c p m -> p c m"))
            wv_sb = consts.tile([P, 4, Dh], BF16)
            nc.sync.dma_start(wv_sb[:], wv[:].rearrange("c p m -> p c m"))
            bqk_sb = consts.tile([P, 1], F32)
            nc.sync.dma_start(bqk_sb[:], bqk[:])
            wproj_sb = consts.tile([P, D], BF16)
            nc.sync.dma_start(wproj_sb[:], wproj[:])
            ident_bf = consts.tile([P, 1], BF16)
            nc.vector.memset(ident_bf[:], 1.0)

            # qk2 rows: 0:64 = qT, 64:128 = kT (matmul-native -> single
            # eviction); kq2 is the partition-swapped copy so row-tiled
            # score pairs find k/q on both PE halves
            qk2 = persist.tile([P, BS], BF16)
            kq2 = persist.tile([P, BS], BF16)
            # padded layout: tile t = [:, t, 63:128]; col 63 = ones (sums row),
            # cols 64:128 = v^T (DMA-transpose needs 128B-aligned dest offsets)
            vaug = persist.tile([P, B * NKT, P], BF16)
            nc.vector.memset(vaug[:], 0.0)
            # ones column at 32 -> softmax sums land on x_ps row 32 (the
            # 32-aligned base PSUM reads need); v^T occupies cols 64:128
            nc.vector.memset(vaug[:, :, 32:33], 1.0)
            # [128, 8 x 512]: col-tiled pairs stacked on 128 partitions
            vt_all = persist.tile([P, B * 2, 512], BF16)

            if skel:
                p_fix = persist.tile([P, NKT, 512], BF16)
                nc.vector.memset(p_fix[:], 0.01)
                xs_fix = persist.tile([P, 512], BF16)
                nc.vector.memset(xs_fix[:], 0.02)
            if 1 not in phases:
                nc.vector.memset(qk2[:], 0.01)
                nc.vector.memset(kq2[:], 0.01)
                nc.vector.memset(vaug[:], 0.01)
                nc.vector.memset(vaug[:, :, 32:33], 1.0)

            for rep in range(repeat):
                # ---------------- phase 1: qkv projections ----------------
                if 1 in phases:
                 with tc.tile_pool(name="qkps", bufs=4, space="PSUM") as qkps, \
                      tc.tile_pool(name="vtps", bufs=3, space="PSUM") as vtps:
                    for b in range(B):
                        ht = hpool.tile([P, 4, S], BF16)
                        nc.sync.dma_start(ht[:],
                                          hT_re[:, :, b * S : (b + 1) * S])
                        for sci in range(4):
                            csl = slice(sci * 512, (sci + 1) * 512)
                            col0 = b * S + sci * 512

                            qk_ps = qkps.tile([P, 512], F32)
                            for c in range(4):
                                nc.tensor.matmul(qk_ps[:], wqk_sb[:, c, :],
                                                 ht[:, c, csl],
                                                 start=(c == 0), stop=(c == 3))
                            sl = slice(col0, col0 + 512)
                            # ONE [128,512] eviction: bias vector already has
                            # q-bias rows 0:64 and k-bias rows 64:128
                            nc.vector.tensor_scalar_add(
                                qk2[:, sl], qk_ps[:], bqk_sb[:])

                        # v: col-tiled M=64 pairs -- chunk 2p -> partitions
                        # 0:64, chunk 2p+1 -> partitions 64:128, concurrent
                        for pair in range(2):
                            sla = slice(pair * 1024, pair * 1024 + 512)
                            slb = slice(pair * 1024 + 512, pair * 1024 + 1024)
                            vt_ps = vtps.tile([P, 512], F32)
                            for c in range(4):
                                nc.tensor.matmul(vt_ps[0:Dh, :], wv_sb[:, c, :],
                                                 ht[:, c, sla],
                                                 start=(c == 0), stop=(c == 3),
                                                 skip_group_check=True)
                                nc.tensor.matmul(vt_ps[Dh:P, :], wv_sb[:, c, :],
                                                 ht[:, c, slb],
                                                 start=(c == 0), stop=(c == 3),
                                                 skip_group_check=True)
                            # one [128,512] eviction covers both col-tiled
                            # chunks (rows 0:64 = chunk a, 64:128 = chunk b)
                            nc.vector.tensor_copy(
                                vt_all[:, b * 2 + pair, :], vt_ps[:])

                        bsl = slice(b * S, (b + 1) * S)
                        nc.sync.dma_start(kq2[0:Dh, bsl], qk2[Dh:P, bsl])
                        nc.sync.dma_start(kq2[Dh:P, bsl], qk2[0:Dh, bsl])
                    if p1tpos == "end":
                        # all transposes back-to-back: a single XBAR-mode
                        # transition on the DMA path instead of one per chunk.
                        # pair j covers chunks 2j (rows 0:64) / 2j+1 (64:128)
                        for j in range(B * 2):
                            nc.sync.dma_start(
                                vaug[:, j * 8 : j * 8 + 4, Dh:P],
                                vt_all[0:Dh, j, :], transpose=True)
                            nc.sync.dma_start(
                                vaug[:, j * 8 + 4 : j * 8 + 8, Dh:P],
                                vt_all[Dh:P, j, :], transpose=True)

                # ---------------- phase 2: attention + proj ----------------
                if 2 in phases:
                 with tc.tile_pool(name="ppool", bufs=pbufs) as ppool, \
                      tc.tile_pool(name="upool", bufs=2) as upool, \
                      tc.tile_pool(name="xspool", bufs=2) as xspool, \
                      tc.tile_pool(name="rsbpool", bufs=2) as rsbpool, \
                      tc.tile_pool(name="recpool", bufs=2) as recpool, \
                      tc.tile_pool(name="outpool", bufs=2) as outpool, \
                      tc.tile_pool(name="spool", bufs=(1 if sfd == 2048 else 2), space="PSUM") as spool, \
                      tc.tile_pool(name="xpool", bufs=2, space="PSUM") as xpool, \
                      tc.tile_pool(name="ops", bufs=2, space="PSUM") as ops:
                    # software-pipelined issue order: attn@V lags its
                    # score-group by AV_LAG so the in-order PE queue never
                    # waits on ACT/DVE; each block's normalize+proj tail is
                    # issued one full block later for the same reason.
                    AV_LAG = av_lag

                    def emit_tail(x_ps, qb, b):
                        # x_ps rows: 0:63 zero, 63 sums, 64:128 attn@V out.
                        # Evict UNNORMALIZED xs (bf16); 1/sum is applied
                        # per-partition at the proj output instead.
                        # ONE full-height eviction: xs row 32 = sums, rows
                        # 64:128 = attn@V out, everything else exact zeros
                        # (vaug cols 0:32+33:64 are zero). wproj_aug rows
                        # [0s;brow;0s;W_h] make the K=128 proj equivalent.
                        xs_t = xspool.tile([P, 512], BF16)
                        nc.vector.tensor_copy(xs_t[:], x_ps[:])
                        # transpose the sums row [1,512] -> [128,4] via 4 PE
                        # transposes into the now-dead x_ps bank (bf16 view,
                        # even columns for 4B-aligned PSUM writes)
                        tp = x_ps[:, 0:4].bitcast(BF16)  # [128, 8] bf16 view
                        for m in range(4):
                            nc.tensor.matmul(
                                tp[:, 2 * m : 2 * m + 1],
                                xs_t[32:33, m * P : (m + 1) * P],
                                ident_bf[32:33, :],
                                is_transpose=True, start=True, stop=True,
                                skip_group_check=True)
                        recip_t = recpool.tile([P, 4], F32)
                        nc.vector.reciprocal(
                            recip_t[:],
                            tp.rearrange("p (a b) -> p a b", b=2)[:, :, 0])
                        out_sb = outpool.tile([P, 4, 512], F16)
                        for m in range(4):
                            out_ps = ops.tile([P, 512], F32, tag="ops")
                            nc.tensor.matmul(out_ps[:],
                                             xs_t[:, m * P : (m + 1) * P],
                                             wproj_sb[:], start=True, stop=True)
                            nc.vector.tensor_scalar_mul(
                                out_sb[:, m, :], out_ps[:],
                                recip_t[:, m : m + 1])
                        row0 = b * S + qb * 512
                        nc.sync.dma_start(
                            part[row0 : row0 + 512, :].rearrange(
                                "(m p) d -> p m d", p=P),
                            out_sb[:])

                    pending_tail = None
                    for qb in range(NQB):
                        if eb == 1:
                            ea_t = eapool.tile([P, NKT, 512], BF16)
                            nc.sync.dma_start(ea_t[:], ea[0, qb])
                        for b in range(B):
                            if eb != 1:
                                ea_t = eapool.tile([P, NKT, 512], BF16)
                                nc.sync.dma_start(ea_t[:], ea[b, qb])
                            qsl = slice(b * S + qb * 512, b * S + (qb + 1) * 512)
                            x_ps = xpool.tile([P, 512], F32)
                            p_all = ppool.tile([P, NKT, 512], BF16)

                            def attn_v(g, first, last):
                                for j in range(2):
                                    tk = g * 2 + j
                                    t = b * NKT + tk
                                    nc.tensor.matmul(
                                        x_ps[:], vaug[:, t, :],
                                        (p_fix if skel else p_all)[:, tk, :],
                                        start=(first and j == 0),
                                        stop=(last and j == 1),
                                        skip_group_check=True)

                            NG = NKT // 2
                            for g in range(NG):
                                if sfd == 1024:
                                    s_ps = spool.tile([P, 1024], F32)
                                    soff = 0
                                else:
                                    # one [128,2048] tile = two score groups
                                    # per exp call, single-buffered (4 banks)
                                    if g % 2 == 0:
                                        s_ps = spool.tile([P, 2048], F32,
                                                          name="s2k")
                                    soff = (g % 2) * 1024
                                # row-tiled K=64 pair: j=0 on PE rows 0:63,
                                # j=1 on rows 64:127 -- concurrent
                                assert rowtile, "split layout needs rowtile"
                                for j in range(2):
                                    tk = g * 2 + j
                                    ksl = slice(b * S + tk * P,
                                                b * S + (tk + 1) * P)
                                    if j == 0:
                                        kop, qop = kq2[0:Dh, ksl], \
                                                   qk2[0:Dh, qsl]
                                    else:
                                        kop, qop = qk2[Dh:P, ksl], \
                                                   kq2[Dh:P, qsl]
                                    nc.tensor.matmul(
                                        s_ps[:, soff + j * 512 :
                                             soff + (j + 1) * 512],
                                        kop, qop, start=True, stop=True)
                                if not skel and g < sch:
                                    # Schraudolph fast-exp on DVE: the bf16
                                    # bit pattern of exp(s+a) is round(C1*
                                    # (s+a)+C2) as int16. ea holds a+C2/C1
                                    # as fp16 BITS for these tks; scores
                                    # never leave [-88, 89] so no clamp or
                                    # saturation handling is needed.
                                    psl = p_all[:, 2 * g : 2 * g + 2, :]\
                                        .rearrange("p a b -> p (a b)")
                                    easl = ea_t[:, 2 * g : 2 * g + 2, :]\
                                        .rearrange("p a b -> p (a b)")
                                    u_t = upool.tile([P, 1024], F32)
                                    nc.vector.tensor_add(
                                        u_t[:], s_ps[:], easl.bitcast(F16))
                                    nc.vector.tensor_scalar(
                                        psl.bitcast(I16), u_t[:],
                                        float(SCH_C1), 0.5,
                                        mybir.AluOpType.mult,
                                        mybir.AluOpType.add)
                                elif sfd == 1024 or g % 2 == 1:
                                    eg = g if sfd == 1024 else g - 1
                                    nt = sfd // 512
                                    psl = p_all[:, eg * 2 : eg * 2 + nt, :]\
                                        .rearrange("p a b -> p (a b)")
                                    if not skel:
                                        nc.scalar.activation(psl, s_ps[:],
                                                             ACT.Exp)
                                if not skel and g % 2 == 1 and g >= sch:
                                    # one multiply per two groups: halves the
                                    # DVE op count (+drains)
                                    lo = max(2 * g - 2, 2 * sch)
                                    p2 = p_all[:, lo : 2 * g + 2, :]\
                                        .rearrange("p a b -> p (a b)")
                                    e2 = ea_t[:, lo : 2 * g + 2, :]\
                                        .rearrange("p a b -> p (a b)")
                                    eng = (nc.gpsimd if g < gp_ea
                                           else nc.vector)
                                    eng.tensor_mul(p2, p2, e2)
                                if g >= AV_LAG:
                                    attn_v(g - AV_LAG, first=(g == AV_LAG),
                                           last=False)
                                if g == tail_g and pending_tail is not None:
                                    emit_tail(*pending_tail)
                                    pending_tail = None
                            for g in range(NG - AV_LAG, NG):
                                attn_v(g, first=False, last=(g == NG - 1))
                            if skel:
                                emit_tail_skel = None
                            pending_tail = (x_ps, qb, b)
                    if pending_tail is not None:
                        emit_tail(*pending_tail)
                        pending_tail = None

    nc.compile()
    return nc


_CACHE = {}


def _get_program(eb: int):
    key = ("prog", eb)
    if key not in _CACHE:
        _CACHE[key] = build_program(eb)
    return _CACHE[key]


def prepare_inputs(hidden_states, attention_mask, alibi_bias, W_qkv, b_qkv,
                   W_proj, b_proj):
    """Host-side prep: transposes, scale folding, exp(alibi), bf16 casts.
    Returns (in_maps, eb)."""
    hidden_states = np.asarray(hidden_states, dtype=np.float32)
    attention_mask = np.asarray(attention_mask)
    alibi_bias = np.asarray(alibi_bias, dtype=np.float32)
    W_qkv = np.asarray(W_qkv, dtype=np.float32)
    b_qkv = np.asarray(b_qkv, dtype=np.float32)
    W_proj = np.asarray(W_proj, dtype=np.float32)
    b_proj = np.asarray(b_proj, dtype=np.float32)

    # per-side scale: scores contract a single K=64 copy (row-tiled pairs)
    s_side = np.float32(np.sqrt(SCALE))

    hiddenT = np.ascontiguousarray(
        hidden_states.reshape(BS, D).T).astype(NP_BF16)

    mask_trivial = bool(attention_mask.all())
    eb = 1 if mask_trivial else B

    def ea_layout(eaT):
        # eaT [S(k), S(q)] -> [NQB, 128, NKT, 512] contiguous per qb slice
        return np.ascontiguousarray(
            eaT.reshape(NKT, P, NQB, 512).transpose(2, 1, 0, 3))

    SCH = 0  # must match build_program's sch default (mask-trivial only)
    ea_all = []
    for h in range(H):
        eaT = np.exp(alibi_bias[0, h].T).astype(NP_BF16)  # [S(k), S(q)]
        if mask_trivial:
            lay = ea_layout(eaT)
            if SCH:
                # additive form for the Schraudolph tks: (a + C2/C1) as
                # fp16, bit-stored in the bf16 tensor
                add = (alibi_bias[0, h].T + SCH_C2_OVER_C1).astype(
                    np.float16).view(NP_BF16)
                lay_add = ea_layout(add)
                lay = lay.copy()
                lay[:, :, 0 : 2 * SCH, :] = lay_add[:, :, 0 : 2 * SCH, :]
            ea_all.append(lay[None])
        else:
            me = np.where(attention_mask, 1.0, 0.0).astype(NP_BF16)  # [B, S]
            ea_all.append(np.stack(
                [ea_layout(eaT * me[bi][:, None]) for bi in range(B)]))
    in_maps = []
    for h in range(H):
        # reference reshapes qkv to (B, S, H, 3*Dh) then splits: head h's
        # q/k/v live in columns [h*3*Dh, h*3*Dh + 3*Dh)
        qs = slice(h * 3 * Dh, h * 3 * Dh + Dh)
        ks = slice(h * 3 * Dh + Dh, h * 3 * Dh + 2 * Dh)
        vs = slice(h * 3 * Dh + 2 * Dh, h * 3 * Dh + 3 * Dh)
        wqk = np.concatenate([W_qkv[:, qs], W_qkv[:, ks]], axis=1) * s_side
        bqk = np.concatenate([b_qkv[qs], b_qkv[ks]]) * s_side
        wv = W_qkv[:, vs]
        Wh = W_proj[h * Dh : (h + 1) * Dh, :]
        # proj bias row: b_proj only on core 0; every core adds bv @ W_h
        # (the v-bias contribution -- sum(attn)=1 makes it a constant)
        brow = (b_proj if h == 0 else np.zeros_like(b_proj)) + b_qkv[vs] @ Wh
        wproj_aug = np.concatenate(
            [np.zeros((32, D), np.float32), brow[None, :],
             np.zeros((31, D), np.float32), Wh], axis=0)
        in_maps.append({
            "hiddenT": hiddenT,
            "ea": ea_all[h],
            "wqk": np.ascontiguousarray(
                wqk.reshape(4, P, P).astype(NP_BF16)),
            "bqk": np.ascontiguousarray(bqk[:, None]),
            "wv": np.ascontiguousarray(wv.reshape(4, P, Dh).astype(NP_BF16)),
            "wproj": wproj_aug.astype(NP_BF16),
        })
    return in_maps, eb


def kernel(**inputs):
    in_maps, eb = prepare_inputs(**inputs)
    nc = _get_program(eb)
    res = run_bass_kernel_spmd(nc, in_maps, list(range(H)))
    out = res.results[0]["part"].astype(np.float32)
    for h in range(1, H):
        out = out + res.results[h]["part"].astype(np.float32)
    return out.reshape(B, S, D)
